# revision 39
# baseline (speedup 1.0000x reference)
"""Trainium2 Bass kernel for nn_Decision_Node (Linear+Hardtanh -> sp, 2-class
softmax Gini -> gini), data-parallel over 8 NeuronCores.

Math per core shard (B_s=128 of B=1024 batches, T=128, F=784, L=256, C=2):
    sp   = clip(x @ W.T + b, -1, 1)                      [N=16384, 256]
    p0   = sigmoid(sp * d),  d = contrib[...,0]-contrib[...,1]
    gini = 2 - p0^2 - p1^2 = 1 + 2 p0 (1-p0) = 1.5 - 0.5*tanh(sp*d/2)^2

Device strategy:
  - x cast to fp16 on host, column-blocked+padded to [7, N, 128] with a
    bias-fold column (x_pad[6,:,16] = 1.0 pairs with wt[6,16,:] = b).
  - fp16 xT tiles loaded with the xbar DMA-transpose (f on partitions),
    fp16 matmuls with fp32 PSUM accumulation (abs err ~1.5e-3).
  - DVE: clip (one fused max/min tensor_scalar) + z = sp*d.
  - ACT: tanh(z/2), square, affine -> gini.
  - 1 MiB batched stores of sp/gini via staging tiles.
"""

import os
import sys
import types
from concurrent.futures import ThreadPoolExecutor

import numpy as np

for _p in (
    "/opt/trn_rl_repo",
    "/root/.axon_site",
    "/root/.axon_site/_ro/trn_rl_repo",
    "/root/.axon_site/_ro/pypackages",
):
    if os.path.isdir(_p) and _p not in sys.path:
        sys.path.append(_p)

B, T, F, L = 1024, 128, 784, 256
NCORES = 8
BS = B // NCORES          # batches per core
NROWS = BS * T            # 16384 rows per core
KT = 7                    # contraction tiles (784 = 6*128 + 16, padded)


def _build_module(nrows, nb, grp):
    """Build + compile the single-core Bass/Tile module (SPMD across cores)."""
    import concourse.tile as tile
    from concourse import bacc, mybir

    f32, f16 = mybir.dt.float32, mybir.dt.float16
    Alu = mybir.AluOpType
    Act = mybir.ActivationFunctionType

    nc = bacc.Bacc(
        "TRN2",
        target_bir_lowering=False,
        debug=False,
        enable_asserts=False,
        num_devices=NCORES,
    )
    KP = 17  # used partitions in the last (remainder+bias) k-tile
    NG = nrows // (grp * 128)  # stage groups total
    xt_d = nc.dram_tensor("xt", [KT, 128, nrows], f16, kind="ExternalInput").ap()
    wt_d = nc.dram_tensor("wt", [KT, 128, L], f16, kind="ExternalInput").ap()
    d_d = nc.dram_tensor("d8", [T, grp * L], f16, kind="ExternalInput").ap()
    # Outputs kept in staged layout [group, partition, grp*L]: every store is
    # a 4 KiB-per-partition linear write; the host de-interleaves on upcast.
    sp_d = nc.dram_tensor("sp", [NG, 128, grp * L], f16, kind="ExternalOutput").ap()
    gi_d = nc.dram_tensor("gini", [NG, 128, grp * L], f16, kind="ExternalOutput").ap()

    blocks = [nb] * (nrows // nb)
    GF = grp * L          # free size of one full stage group (2048)

    with tile.TileContext(nc) as tc:
        with (
            tc.tile_pool(name="consts", bufs=1) as consts,
            tc.tile_pool(name="xt", bufs=4) as xt_pool,
            tc.tile_pool(name="psum", bufs=8, space="PSUM") as psum_pool,
            tc.tile_pool(name="stage", bufs=2) as stage_pool,
            tc.tile_pool(name="tmp", bufs=2) as tmp_pool,
        ):
            wt_sb = consts.tile([128, KT, L], f16)
            nc.scalar.dma_start(wt_sb[:], wt_d.rearrange("k p l -> p k l"))
            d8_sb = consts.tile([128, GF], f16)
            nc.scalar.dma_start(d8_sb[:], d_d[:])
            # Persistent last-k-tile buffers: rows 17..127 stay zero so the
            # matmul can always contract over 128 partitions (keeps FWL on);
            # only the 17 real rows are re-DMAed per block (double-buffered).
            xk6s = []
            for i in range(2):
                xk6 = consts.tile([128, nb], f16, tag=f"xk6_{i}")
                nc.vector.memset(xk6[:], 0.0)
                xk6s.append(xk6)

            n0 = 0
            for bi, bnb in enumerate(blocks):
                tpb = bnb // 128
                last_blk = bi == len(blocks) - 1
                # 2-tile groups in the last block: the tail elementwise+store
                # chain drains in ~0.5 MiB pieces instead of one 2 MiB gulp.
                bgrp = 2 if last_blk else min(grp, tpb)
                xts = []
                for k in range(KT - 1):
                    xk = xt_pool.tile(
                        [128, bnb], f16, tag=f"x{k}", bufs=5 if k < 3 else 4
                    )
                    nc.sync.dma_start(xk[:], xt_d[k, :, n0 : n0 + bnb])
                    xts.append(xk)
                xk6 = xk6s[bi % 2]
                nc.sync.dma_start(xk6[0:KP, :bnb], xt_d[KT - 1, 0:KP, n0 : n0 + bnb])
                xts.append(xk6)
                for g in range(tpb // bgrp):
                    gf = bgrp * L
                    sp_st = stage_pool.tile([128, bgrp, L], f16, tag="sp_st")
                    gi_st = stage_pool.tile([128, bgrp, L], f16, tag="gi_st")
                    z_big = tmp_pool.tile([128, gf], f16, tag="z")
                    for h in range(bgrp):
                        t = g * bgrp + h
                        ps = psum_pool.tile([128, L], f32)
                        for k in range(KT):
                            nc.tensor.matmul(
                                ps[:],
                                xts[k][:, t * 128 : (t + 1) * 128],
                                wt_sb[:, k, :],
                                start=(k == 0),
                                stop=(k == KT - 1),
                            )
                        # fused hardtanh: (ps max -1) min 1, PSUM -> stage
                        nc.vector.tensor_scalar(
                            sp_st[:, h, :],
                            ps[:],
                            -1.0,
                            1.0,
                            Alu.max,
                            Alu.min,
                        )
                    sp_flat = sp_st[:].rearrange("p a l -> p (a l)")
                    nc.vector.tensor_tensor(
                        z_big[:], sp_flat, d8_sb[:, :gf], Alu.mult
                    )
                    th_big = tmp_pool.tile([128, gf], f16, tag="th")
                    nc.scalar.activation(th_big[:], z_big[:], Act.Tanh, scale=0.5)
                    # device stores u = tanh(z/2)^2 in fp16; the host applies
                    # gini = 1.5 - 0.5*u during the fp32 upcast/assembly.
                    nc.vector.tensor_tensor(
                        gi_st[:].rearrange("p a l -> p (a l)"),
                        th_big[:],
                        th_big[:],
                        Alu.mult,
                    )
                    r0 = n0 + g * bgrp * 128
                    gidx, aoff = divmod(r0 // 128, grp)
                    dst_sp = sp_d[gidx, :, aoff * L : (aoff + bgrp) * L]
                    dst_gi = gi_d[gidx, :, aoff * L : (aoff + bgrp) * L]
                    store_eng = nc.sync if last_blk else nc.gpsimd
                    store_eng.dma_start(dst_sp, sp_st[:])
                    store_eng.dma_start(dst_gi, gi_st[:])
                n0 += bnb

    nc.compile()
    return nc


def _prep_core_x(x_flat_core):
    """[16384, 784] fp32 -> transposed fp16 [7, 128, 16384] (f on partitions).

    Row 16 of the last k-tile is the all-ones bias-fold row.
    """
    n = x_flat_core.shape[0]
    xsT16 = x_flat_core.T.astype(np.float16)  # [784, n], one strided pass
    xt = np.zeros((KT, 128, n), np.float16)
    xt[:6] = xsT16[:768].reshape(6, 128, n)
    xt[6, :16] = xsT16[768:784]
    xt[6, 16] = 1.0
    return xt


def _prep_wt(W, b):
    wt = np.zeros((KT, 128, L), np.float16)
    WT = W.T  # [784, 256]
    for k in range(6):
        wt[k] = WT[k * 128 : (k + 1) * 128]
    wt[6, :16] = WT[768:784]
    wt[6, 16] = b
    return wt


_module_cache = {}


def _get_module(nrows, nb, grp):
    key = (nrows, nb, grp)
    if key not in _module_cache:
        _module_cache[key] = _build_module(nrows, nb, grp)
    return _module_cache[key]


def _install_ntff_hook():
    """Register the axon NTFF profiling hook missing from this image's antenv."""
    try:
        import antenv.axon_hooks  # noqa: F401

        return
    except ImportError:
        pass
    try:
        from trn_agent_boot.trn_boot import _ntff_profile_via_ctypes

        hook = _ntff_profile_via_ctypes("/opt/axon/libaxon_pjrt.so")
    except Exception:
        hook = None
    mod = types.ModuleType("antenv.axon_hooks")
    mod.get_axon_ntff_profile_hook = lambda: hook
    mod.set_axon_ntff_profile_hook = lambda h: None
    sys.modules["antenv.axon_hooks"] = mod


def _run(x, W, b, contribution, trace=False, tmpdir=None):
    from concourse import bass_utils

    nc = _get_module(NROWS, 2048, 8)

    x_flat = np.ascontiguousarray(x, dtype=np.float32).reshape(NCORES, NROWS, F)
    wt = _prep_wt(np.asarray(W, np.float32), np.asarray(b, np.float32))
    c = np.asarray(contribution, np.float32)
    d = np.ascontiguousarray(c[:, :, 0] - c[:, :, 1], dtype=np.float32)
    d8 = np.ascontiguousarray(np.tile(d, (1, 8)).astype(np.float16))

    with ThreadPoolExecutor(NCORES) as ex:
        xts = list(ex.map(_prep_core_x, [x_flat[i] for i in range(NCORES)]))

    if trace:
        _install_ntff_hook()
    in_maps = [{"xt": xts[i], "wt": wt, "d8": d8} for i in range(NCORES)]
    res = bass_utils.run_bass_kernel_spmd(
        nc, in_maps, core_ids=list(range(NCORES)), trace=trace, tmpdir=tmpdir
    )

    def _unstage(raw):
        # [NG, 128, 8*256] staged -> [nrows, 256] row-major
        ng = raw.shape[0]
        return raw.reshape(ng, 128, 8, L).swapaxes(1, 2).reshape(ng * 1024, L)

    sp = np.concatenate([_unstage(res.results[i]["sp"]) for i in range(NCORES)])
    u = np.concatenate([_unstage(res.results[i]["gini"]) for i in range(NCORES)])
    gini = 1.5 - 0.5 * u.reshape(B, T, L).astype(np.float32)
    out = (sp.reshape(B, T, L).astype(np.float32), gini)
    return (out, res) if trace else (out, None)


def kernel(x, W, b, contribution):
    out, _ = _run(x, W, b, contribution, trace=False)
    return out


# revision 45
# speedup vs baseline: 1.0387x; 1.0387x over previous
"""Trainium2 Bass kernel for nn_Decision_Node (Linear+Hardtanh -> sp, 2-class
softmax Gini -> gini), data-parallel over 8 NeuronCores.

Math per core shard (B_s=128 of B=1024 batches, T=128, F=784, L=256, C=2):
    sp   = clip(x @ W.T + b, -1, 1)                      [N=16384, 256]
    p0   = sigmoid(sp * d),  d = contrib[...,0]-contrib[...,1]
    gini = 2 - p0^2 - p1^2 = 1 + 2 p0 (1-p0) = 1.5 - 0.5*tanh(sp*d/2)^2

Device strategy:
  - x cast to fp16 on host, column-blocked+padded to [7, N, 128] with a
    bias-fold column (x_pad[6,:,16] = 1.0 pairs with wt[6,16,:] = b).
  - fp16 xT tiles loaded with the xbar DMA-transpose (f on partitions),
    fp16 matmuls with fp32 PSUM accumulation (abs err ~1.5e-3).
  - DVE: clip (one fused max/min tensor_scalar) + z = sp*d.
  - ACT: tanh(z/2), square, affine -> gini.
  - 1 MiB batched stores of sp/gini via staging tiles.
"""

import os
import sys
import types
from concurrent.futures import ThreadPoolExecutor

import numpy as np

for _p in (
    "/opt/trn_rl_repo",
    "/root/.axon_site",
    "/root/.axon_site/_ro/trn_rl_repo",
    "/root/.axon_site/_ro/pypackages",
):
    if os.path.isdir(_p) and _p not in sys.path:
        sys.path.append(_p)

B, T, F, L = 1024, 128, 784, 256
NCORES = 8
BS = B // NCORES          # batches per core
NROWS = BS * T            # 16384 rows per core
KT = 7                    # contraction tiles (784 = 6*128 + 16, padded)


def _build_module(nrows, nb, grp):
    """Build + compile the single-core Bass/Tile module (SPMD across cores)."""
    import concourse.tile as tile
    from concourse import bacc, mybir

    f32, f16 = mybir.dt.float32, mybir.dt.float16
    Alu = mybir.AluOpType
    Act = mybir.ActivationFunctionType

    nc = bacc.Bacc(
        "TRN2",
        target_bir_lowering=False,
        debug=False,
        enable_asserts=False,
        num_devices=NCORES,
    )
    KP = 17  # used partitions in the last (remainder+bias) k-tile
    NG = nrows // (grp * 128)  # stage groups total
    xt_d = nc.dram_tensor("xt", [KT, 128, nrows], f16, kind="ExternalInput").ap()
    wt_d = nc.dram_tensor("wt", [KT, 128, L], f16, kind="ExternalInput").ap()
    d_d = nc.dram_tensor("d8", [T, grp * L], f16, kind="ExternalInput").ap()
    # Outputs kept in staged layout [group, partition, grp*L]: every store is
    # a 4 KiB-per-partition linear write; the host de-interleaves on upcast.
    sp_d = nc.dram_tensor("sp", [NG, 128, grp * L], f16, kind="ExternalOutput").ap()
    # gini intermediate u = tanh^2 in [0,1] quantized to uint8 (u8 = 255u+0.5,
    # truncating cast => round-to-nearest); host reconstructs 1.5 - 0.5*u8/255.
    gi_d = nc.dram_tensor(
        "gini", [NG, 128, grp * L], mybir.dt.uint8, kind="ExternalOutput"
    ).ap()

    blocks = [nb] * (nrows // nb)
    GF = grp * L          # free size of one full stage group (2048)

    with tile.TileContext(nc) as tc:
        with (
            tc.tile_pool(name="consts", bufs=1) as consts,
            tc.tile_pool(name="xt", bufs=4) as xt_pool,
            tc.tile_pool(name="psum", bufs=8, space="PSUM") as psum_pool,
            tc.tile_pool(name="stage", bufs=2) as stage_pool,
            tc.tile_pool(name="tmp", bufs=2) as tmp_pool,
        ):
            wt_sb = consts.tile([128, KT, L], f16)
            nc.scalar.dma_start(wt_sb[:], wt_d.rearrange("k p l -> p k l"))
            d8_sb = consts.tile([128, GF], f16)
            nc.scalar.dma_start(d8_sb[:], d_d[:])
            # Persistent last-k-tile buffers: rows 17..127 stay zero so the
            # matmul can always contract over 128 partitions (keeps FWL on);
            # only the 17 real rows are re-DMAed per block (double-buffered).
            xk6s = []
            for i in range(2):
                xk6 = consts.tile([128, nb], f16, tag=f"xk6_{i}")
                nc.vector.memset(xk6[:], 0.0)
                xk6s.append(xk6)

            n0 = 0
            for bi, bnb in enumerate(blocks):
                tpb = bnb // 128
                bgrp = min(grp, tpb)   # tiles per stage group in this block
                xts = []
                for k in range(KT - 1):
                    xk = xt_pool.tile(
                        [128, bnb], f16, tag=f"x{k}", bufs=5 if k < 3 else 4
                    )
                    nc.sync.dma_start(xk[:], xt_d[k, :, n0 : n0 + bnb])
                    xts.append(xk)
                xk6 = xk6s[bi % 2]
                nc.sync.dma_start(xk6[0:KP, :bnb], xt_d[KT - 1, 0:KP, n0 : n0 + bnb])
                xts.append(xk6)
                for g in range(tpb // bgrp):
                    gf = bgrp * L
                    sp_st = stage_pool.tile([128, bgrp, L], f16, tag="sp_st")
                    gi_st = stage_pool.tile(
                        [128, bgrp, L], mybir.dt.uint8, tag="gi_st"
                    )
                    z_big = tmp_pool.tile([128, gf], f16, tag="z")
                    for h in range(bgrp):
                        t = g * bgrp + h
                        ps = psum_pool.tile([128, L], f32)
                        for k in range(KT):
                            nc.tensor.matmul(
                                ps[:],
                                xts[k][:, t * 128 : (t + 1) * 128],
                                wt_sb[:, k, :],
                                start=(k == 0),
                                stop=(k == KT - 1),
                            )
                        # fused hardtanh: (ps max -1) min 1, PSUM -> stage
                        nc.vector.tensor_scalar(
                            sp_st[:, h, :],
                            ps[:],
                            -1.0,
                            1.0,
                            Alu.max,
                            Alu.min,
                        )
                    sp_flat = sp_st[:].rearrange("p a l -> p (a l)")
                    nc.vector.tensor_tensor(
                        z_big[:], sp_flat, d8_sb[:, :gf], Alu.mult
                    )
                    th_big = tmp_pool.tile([128, gf], f16, tag="th")
                    nc.scalar.activation(th_big[:], z_big[:], Act.Tanh, scale=0.5)
                    th2_big = tmp_pool.tile([128, gf], f16, tag="th2")
                    nc.vector.tensor_tensor(
                        th2_big[:], th_big[:], th_big[:], Alu.mult
                    )
                    nc.vector.tensor_scalar(
                        gi_st[:].rearrange("p a l -> p (a l)"),
                        th2_big[:],
                        255.0,
                        0.5,
                        Alu.mult,
                        Alu.add,
                    )
                    gidx = (n0 + g * bgrp * 128) // (grp * 128)
                    nc.gpsimd.dma_start(sp_d[gidx], sp_st[:])
                    nc.gpsimd.dma_start(gi_d[gidx], gi_st[:])
                n0 += bnb

    nc.compile()
    return nc


def _prep_core_x(x_flat_core):
    """[16384, 784] fp32 -> transposed fp16 [7, 128, 16384] (f on partitions).

    Row 16 of the last k-tile is the all-ones bias-fold row.
    """
    n = x_flat_core.shape[0]
    xsT16 = x_flat_core.T.astype(np.float16)  # [784, n], one strided pass
    xt = np.zeros((KT, 128, n), np.float16)
    xt[:6] = xsT16[:768].reshape(6, 128, n)
    xt[6, :16] = xsT16[768:784]
    xt[6, 16] = 1.0
    return xt


def _prep_wt(W, b):
    wt = np.zeros((KT, 128, L), np.float16)
    WT = W.T  # [784, 256]
    for k in range(6):
        wt[k] = WT[k * 128 : (k + 1) * 128]
    wt[6, :16] = WT[768:784]
    wt[6, 16] = b
    return wt


_module_cache = {}


def _get_module(nrows, nb, grp):
    key = (nrows, nb, grp)
    if key not in _module_cache:
        _module_cache[key] = _build_module(nrows, nb, grp)
    return _module_cache[key]


def _install_ntff_hook():
    """Register the axon NTFF profiling hook missing from this image's antenv."""
    try:
        import antenv.axon_hooks  # noqa: F401

        return
    except ImportError:
        pass
    try:
        from trn_agent_boot.trn_boot import _ntff_profile_via_ctypes

        hook = _ntff_profile_via_ctypes("/opt/axon/libaxon_pjrt.so")
    except Exception:
        hook = None
    mod = types.ModuleType("antenv.axon_hooks")
    mod.get_axon_ntff_profile_hook = lambda: hook
    mod.set_axon_ntff_profile_hook = lambda h: None
    sys.modules["antenv.axon_hooks"] = mod


def _run(x, W, b, contribution, trace=False, tmpdir=None):
    from concourse import bass_utils

    nc = _get_module(NROWS, 2048, 8)

    x_flat = np.ascontiguousarray(x, dtype=np.float32).reshape(NCORES, NROWS, F)
    wt = _prep_wt(np.asarray(W, np.float32), np.asarray(b, np.float32))
    c = np.asarray(contribution, np.float32)
    d = np.ascontiguousarray(c[:, :, 0] - c[:, :, 1], dtype=np.float32)
    d8 = np.ascontiguousarray(np.tile(d, (1, 8)).astype(np.float16))

    with ThreadPoolExecutor(NCORES) as ex:
        xts = list(ex.map(_prep_core_x, [x_flat[i] for i in range(NCORES)]))

    if trace:
        _install_ntff_hook()
    in_maps = [{"xt": xts[i], "wt": wt, "d8": d8} for i in range(NCORES)]
    res = bass_utils.run_bass_kernel_spmd(
        nc, in_maps, core_ids=list(range(NCORES)), trace=trace, tmpdir=tmpdir
    )

    def _unstage(raw):
        # [NG, 128, 8*256] staged -> [nrows, 256] row-major
        ng = raw.shape[0]
        return raw.reshape(ng, 128, 8, L).swapaxes(1, 2).reshape(ng * 1024, L)

    sp = np.concatenate([_unstage(res.results[i]["sp"]) for i in range(NCORES)])
    u = np.concatenate([_unstage(res.results[i]["gini"]) for i in range(NCORES)])
    gini = 1.5 - (0.5 / 255.0) * u.reshape(B, T, L).astype(np.float32)
    out = (sp.reshape(B, T, L).astype(np.float32), gini)
    return (out, res) if trace else (out, None)


def kernel(x, W, b, contribution):
    out, _ = _run(x, W, b, contribution, trace=False)
    return out
